# revision 20
# baseline (speedup 1.0000x reference)
"""Trainium2 Bass kernel for LocalWindowAttention.

Reference semantics (per batch b):
    pad seq 4000 -> 4096, split into 32 windows of 128 tokens.
    qkv = x @ w_qkv.T + b_qkv ; per-window per-head softmax(q k^T / sqrt(64)) @ v
    out = o @ w_out.T + b_out ; drop padded tail.

Sharding: data-parallel over batch. Core b computes batch b fully.

Per-core layout strategy (everything chosen so matmul contraction = partition dim):
  - x is staged feature-major  xT[e, t]  (e on partitions, 8 chunks of 128).
  - Q computed feature-major (f on partitions); K likewise but stored per-head
    zero-padded to the full 128 partitions (kz) so every score matmul reads
    inputs at base partition 0 (mixing base partitions 0/64 across matmuls
    crashes the runtime):
        S^T[tk, tq] = sum_d K[d, tk] Q[d, tq]   (lhsT=kz_h, rhs=Q pair, K=128)
  - V computed token-major (t on partitions) so AV works with V as stationary:
        O_u[d, tq] = sum_tk V[tk, d] E[tk, tq]
  - softmax denominators: 16 accumulating matmuls with one-hot selectors
        D16[h, tq] = sum_tk E[tk, h*128+tq]  -> reciprocal_approx_fast
    broadcast back to O shape via a (16 x 128) selector matmul, then one DVE
    multiply normalizes O.  (1/sqrt(64) is folded into w_q on the host; exp is
    computed without max-subtraction which is exact for softmax and safe here:
    |scores| <= ~3.)
  - out projection consumes O feature-major chunks directly.
All matmuls use bf16/fp16 operands (1 cycle/row on TRN2; fp32 is 4x slower).
Accumulation is always fp32 in PSUM.
"""

import sys
import numpy as np

for _p in ("/opt/trn_rl_repo", "/root/.axon_site/_ro/trn_rl_repo"):
    if _p not in sys.path:
        sys.path.append(_p)

import ml_dtypes

P = 128          # partitions
E = 1024         # embed dim
H = 16           # heads
D = 64           # head dim
W = 128          # window
B = 8            # batch
S = 4000         # seq len
SP = 4096        # padded seq len
NW = SP // W     # 32 windows
CW = 4           # windows per chunk
CT = CW * W      # 512 tokens per chunk
EC = 8           # e-chunks of 128

BF16 = ml_dtypes.bfloat16
F16 = np.float16

_cache = {}


def build_nc(n_chunks, s_out, has_bqk, has_bout):
    """Build + compile the single-core Bass program (same program for all cores)."""
    from concourse import bacc, tile, mybir

    dt = mybir.dt
    AF = mybir.ActivationFunctionType

    nc = bacc.Bacc(None, target_bir_lowering=False, debug=False)

    xt_d = nc.dram_tensor("xt", [n_chunks, P, EC, CT], dt.bfloat16, kind="ExternalInput")
    wqkv_d = nc.dram_tensor("wqkv", [P, EC, 3 * E], dt.bfloat16, kind="ExternalInput")
    wout_d = nc.dram_tensor("wout", [P, EC, E], dt.bfloat16, kind="ExternalInput")
    oh_d = nc.dram_tensor("onehot", [P, H, H], dt.bfloat16, kind="ExternalInput")
    sel_d = nc.dram_tensor("sel", [H, EC, P], dt.float16, kind="ExternalInput")
    out_d = nc.dram_tensor("out", [s_out, E], dt.float32, kind="ExternalOutput")
    if has_bqk:
        bqk_d = nc.dram_tensor("bqk", [P, 2, EC], dt.float32, kind="ExternalInput")
    if has_bout:
        cb_d = nc.dram_tensor("cb", [P, 2, 512], dt.float32, kind="ExternalInput")

    with tile.TileContext(nc) as tc:
        with (
            tc.tile_pool(name="const", bufs=1) as constp,
            tc.tile_pool(name="xp", bufs=2) as xp,
            tc.tile_pool(name="qkp", bufs=2) as qkp,
            tc.tile_pool(name="kzp", bufs=1) as kzp,
            tc.tile_pool(name="ktp", bufs=3) as ktp,
            tc.tile_pool(name="vp", bufs=2) as vp,
            tc.tile_pool(name="ep", bufs=2) as ep,
            tc.tile_pool(name="op", bufs=2) as opool,
            tc.tile_pool(name="rp", bufs=2) as rp,
            tc.tile_pool(name="fpl", bufs=3) as fpl,
            tc.tile_pool(name="psA", bufs=4, space="PSUM") as psA,
        ):
            oh = constp.tile([P, H, H], dt.bfloat16)
            nc.sync.dma_start(oh[:], oh_d[:])
            sel = constp.tile([H, EC, P], dt.float16)
            nc.sync.dma_start(sel[:], sel_d[:])
            # weights split per e-chunk so the first QKV matmul (which only
            # needs wq[:, 0]) isn't gated on the full 8.4MB transfer
            wq = constp.tile([P, EC, 3 * E], dt.bfloat16)
            for ec in range(EC):
                nc.sync.dma_start(wq[:, ec, :], wqkv_d[:, ec, :])
            wo = constp.tile([P, EC, E], dt.bfloat16)
            for ec in range(EC):
                nc.sync.dma_start(wo[:, ec, :], wout_d[:, ec, :])
            if has_bqk:
                bqk = constp.tile([P, 2, EC], dt.float32)
                nc.sync.dma_start(bqk[:], bqk_d[:])
            if has_bout:
                cb = constp.tile([P, 2, 512], dt.float32)
                nc.sync.dma_start(cb[:], cb_d[:])

            # kz zero halves never change: clear the two persistent tiles once.
            kz_tiles = []
            for i in range(2):
                kzt = kzp.tile([P, H, CT], dt.bfloat16, tag=f"kz{i}", name=f"kz{i}")
                nc.gpsimd.memset(kzt[:], 0.0)
                kz_tiles.append(kzt)

            def stage_a1(wi, kz_sb, q_sb):
                """scores -> exp (quartered ACTs so D16 can start early)."""
                e_sb = ep.tile([P, H, W], dt.bfloat16, tag="e")
                for half in range(2):
                    ps_s = psA.tile([P, 8, W], dt.float32, tag="ps")
                    for j in range(8):
                        h = half * 8 + j
                        # kz's invalid half is zero, so contracting all 128
                        # rows against the Q f-tile pair selects head h.
                        nc.tensor.matmul(
                            ps_s[:, j, :],
                            kz_sb[:, h, wi * W:(wi + 1) * W],
                            q_sb[:, h // 2, wi * W:(wi + 1) * W],
                            start=True,
                            stop=True,
                        )
                    for qq in range(2):
                        nc.scalar.activation(
                            e_sb[:, half * 8 + qq * 4:half * 8 + qq * 4 + 4, :],
                            ps_s[:, qq * 4:qq * 4 + 4, :], AF.Exp,
                        )
                return e_sb

            def stage_a2_d16(e_sb):
                """denominators D16[h, tq] via accumulating one-hot matmuls."""
                ps_d = psA.tile([H, W], dt.float32, tag="ps")
                for h in range(H):
                    nc.tensor.matmul(
                        ps_d[:],
                        oh[:, h, :],
                        e_sb[:, h, :],
                        start=(h == 0),
                        stop=(h == H - 1),
                    )
                return ps_d

            def stage_a2(wi, ps_d, e_sb, v_sb):
                """recip -> broadcast -> AV -> normalized O."""
                rd32 = rp.tile([H, W], dt.float32, tag="rd32")
                nc.vector.reciprocal_approx_fast(rd32[:], ps_d[:])
                rd16 = rp.tile([H, W], dt.float16, tag="rd16")
                nc.vector.tensor_copy(rd16[:], rd32[:])

                # broadcast recip to O shape: R_O[cc*128+p, tq] = rd[2cc+p//64, tq]
                ps_r = psA.tile([P, EC, W], dt.float32, tag="ps")
                for cc in range(EC):
                    nc.tensor.matmul(
                        ps_r[:, cc, :], sel[:, cc, :], rd16[:],
                        start=True, stop=True,
                    )
                # evict R_O to sbuf right away (runs during the AV matmuls, so
                # the post-AV critical chain is just one multiply per half)
                r_sb = rp.tile([P, EC, W], dt.float16, tag="ro")
                nc.vector.tensor_copy(r_sb[:], ps_r[:])

                # unnormalized O_u[d, tq] per head (2 heads per 128-row chunk)
                ps_o = psA.tile([P, EC, W], dt.float32, tag="ps")
                for h in range(H):
                    cc = h // 2
                    po = (h % 2) * D
                    nc.tensor.matmul(
                        ps_o[po:po + D, cc, :],
                        v_sb[:, wi, h // 8, (h % 8) * D:(h % 8) * D + D],
                        e_sb[:, h, :],
                        start=True,
                        stop=True,
                    )
                # normalize in halves into separate tiles so the out-projection
                # can start as soon as the first half is multiplied
                o_halves = []
                for hh in range(2):
                    o_h = opool.tile([P, 4, W], dt.bfloat16, tag=f"o{hh}",
                                     name=f"o{hh}")
                    sl = slice(hh * 4, hh * 4 + 4)
                    nc.vector.tensor_mul(o_h[:], ps_o[:, sl, :], r_sb[:, sl, :])
                    o_halves.append(o_h)
                return o_halves

            def stage_b_mm(ps_f, o_halves, fh):
                """one half of the out projection accumulation"""
                for cc in range(EC):
                    nc.tensor.matmul(
                        ps_f[:, fh, :],
                        o_halves[cc // 4][:, cc % 4, :],
                        wo[:, cc, fh * 512:(fh + 1) * 512],
                        start=(cc == 0),
                        stop=(cc == EC - 1),
                    )

            def stage_b_out(ps_f, row0, rows):
                f_sb = fpl.tile([P, 2, 512], dt.float32, tag="f")
                if has_bout:
                    nc.vector.tensor_add(f_sb[:], ps_f[:], cb[:])
                else:
                    # on ScalarE: keeps DVE free for the recip/normalize chain
                    nc.scalar.activation(f_sb[:], ps_f[:], AF.Copy)
                nc.sync.dma_start(out_d[row0:row0 + rows, :], f_sb[:rows])

            pend = None
            for c in range(n_chunks):
                xt = xp.tile([P, EC, CT], dt.bfloat16, tag="xt")
                nc.sync.dma_start(xt[:], xt_d[c])

                q_sb = qkp.tile([P, EC, CT], dt.bfloat16, tag="q")
                kz_sb = kz_tiles[c % 2]
                v_sb = vp.tile([P, CW, 2, 512], dt.bfloat16, tag="v")

                # ---- Q and K (feature-major): psum[f_tile, t] ----
                for which in (0, 1):
                    for fg in range(4):  # pairs of f-tiles -> one 2-bank psum tile
                        ps = psA.tile([P, 2, 512], dt.float32, tag="ps")
                        for half in range(2):
                            ft = fg * 2 + half
                            off = which * E + ft * P
                            for ec in range(EC):
                                nc.tensor.matmul(
                                    ps[:, half, :],
                                    wq[:, ec, off:off + P],
                                    xt[:, ec, :],
                                    start=(ec == 0),
                                    stop=(ec == EC - 1),
                                )
                        if which == 0:  # Q: keep f-tile-major pair layout
                            if has_bqk:
                                for half in range(2):
                                    ft = fg * 2 + half
                                    nc.scalar.activation(
                                        q_sb[:, ft, :], ps[:, half, :], AF.Identity,
                                        bias=bqk[:, 0, ft:ft + 1],
                                    )
                            else:
                                nc.scalar.activation(
                                    q_sb[:, fg * 2:fg * 2 + 2, :], ps[:], AF.Copy,
                                )
                        else:  # K: evict pair once, DMA-scatter into kz halves
                            ktmp = ktp.tile([P, 2, 512], dt.bfloat16, tag="kt")
                            if has_bqk:
                                for half in range(2):
                                    ft = fg * 2 + half
                                    nc.scalar.activation(
                                        ktmp[:, half, :], ps[:, half, :],
                                        AF.Identity, bias=bqk[:, 1, ft:ft + 1],
                                    )
                            else:
                                nc.scalar.activation(ktmp[:], ps[:], AF.Copy)
                            for half in range(2):
                                ft = fg * 2 + half
                                for hh in range(2):
                                    pr = slice(hh * 64, hh * 64 + 64)
                                    nc.sync.dma_start(
                                        kz_sb[pr, 2 * ft + hh, :],
                                        ktmp[pr, half, :],
                                    )

                # ---- V (token-major): psum[t, f] per window ----
                for wi in range(CW):
                    ps = psA.tile([P, 2, 512], dt.float32, tag="ps")
                    for fh in range(2):
                        off = 2 * E + fh * 512
                        for ec in range(EC):
                            nc.tensor.matmul(
                                ps[:, fh, :],
                                xt[:, ec, wi * W:(wi + 1) * W],
                                wq[:, ec, off:off + 512],
                                start=(ec == 0),
                                stop=(ec == EC - 1),
                            )
                    nc.vector.tensor_copy(v_sb[:, wi], ps[:])

                # ---- attention (A) + out-projection (B), software-pipelined:
                # B(w) is emitted after A(w+1) so the PE has score/AV matmuls
                # to run while w's evict->normalize chain goes through ACT/DVE.
                for wi in range(CW):
                    g = c * CW + wi
                    row0 = g * W
                    rows = min(s_out - row0, W)
                    if rows <= 0:
                        continue
                    e_sb = stage_a1(wi, kz_sb, q_sb)
                    if pend is not None:  # outproj half 0 of w-1 covers exp(w)
                        ps_f = psA.tile([P, 2, 512], dt.float32, tag="ps")
                        stage_b_mm(ps_f, pend[0], 0)
                    ps_d = stage_a2_d16(e_sb)
                    if pend is not None:  # half 1 covers recip/cast chain
                        stage_b_mm(ps_f, pend[0], 1)
                        stage_b_out(ps_f, pend[1], pend[2])
                    o_halves = stage_a2(wi, ps_d, e_sb, v_sb)
                    pend = (o_halves, row0, rows)

            if pend is not None:
                ps_f = psA.tile([P, 2, 512], dt.float32, tag="ps")
                stage_b_mm(ps_f, pend[0], 0)
                stage_b_mm(ps_f, pend[0], 1)
                stage_b_out(ps_f, pend[1], pend[2])

    nc.compile()
    return nc


def prep_inputs(x, w_qkv, b_qkv, w_out, b_out, n_chunks, s_out):
    """Host-side staging: pad, transpose, cast, fold scale into w_q."""
    sp = n_chunks * CT
    nb = x.shape[0]

    wqkvT = np.ascontiguousarray(w_qkv.T).astype(np.float32).copy()
    wqkvT[:, :E] *= 1.0 / np.sqrt(D)
    wqkv_sb = np.ascontiguousarray(
        wqkvT.reshape(EC, P, 3 * E).transpose(1, 0, 2)
    ).astype(BF16)

    woutT = np.ascontiguousarray(w_out.T)
    wout_sb = np.ascontiguousarray(
        woutT.reshape(EC, P, E).transpose(1, 0, 2)
    ).astype(BF16)

    oh = np.zeros((P, H, H), dtype=BF16)
    for h in range(H):
        oh[:, h, h] = 1
    selm = np.zeros((H, EC, P), dtype=F16)
    for cc in range(EC):
        for m in range(P):
            selm[2 * cc + m // D, cc, m] = 1

    base = {"wqkv": wqkv_sb, "wout": wout_sb, "onehot": oh, "sel": selm}

    has_bqk = bool(np.any(b_qkv[:2 * E]))
    has_bout = bool(np.any(b_out)) or bool(np.any(b_qkv[2 * E:]))
    if has_bqk:
        bqk = np.stack(
            [b_qkv[:E].reshape(EC, P).T / np.sqrt(D),
             b_qkv[E:2 * E].reshape(EC, P).T], axis=1
        ).astype(np.float32)  # (P, 2, EC)
        base["bqk"] = np.ascontiguousarray(bqk)
    if has_bout:
        cbv = (b_out + b_qkv[2 * E:] @ w_out.T).astype(np.float32)  # (E,)
        base["cb"] = np.ascontiguousarray(
            np.broadcast_to(cbv.reshape(1, 2, 512), (P, 2, 512))
        ).copy()

    in_maps = []
    for b in range(nb):
        xp_ = np.zeros((sp, E), dtype=np.float32)
        xp_[:min(s_out, x.shape[1])] = x[b][:s_out]
        xT = np.ascontiguousarray(xp_.T)  # (E, sp)
        xt_sb = np.ascontiguousarray(
            xT.reshape(EC, P, n_chunks, CT).transpose(2, 1, 0, 3)
        ).astype(BF16)  # (n_chunks, P, EC, CT)
        m = dict(base)
        m["xt"] = xt_sb
        in_maps.append(m)
    return in_maps, has_bqk, has_bout


def run(x, w_qkv, b_qkv, w_out, b_out, n_chunks=NW // CW, s_out=S, trace=False):
    from concourse import bass_utils

    in_maps, has_bqk, has_bout = prep_inputs(
        x, w_qkv, b_qkv, w_out, b_out, n_chunks, s_out
    )
    key = (n_chunks, s_out, has_bqk, has_bout)
    if key not in _cache:
        _cache[key] = build_nc(*key)
    nc = _cache[key]

    res = bass_utils.run_bass_kernel_spmd(
        nc, in_maps, core_ids=list(range(len(in_maps))), trace=trace,
    )
    out = np.stack([r["out"] for r in res.results], axis=0)
    return out, res


def kernel(x, w_qkv, b_qkv, w_out, b_out):
    x = np.asarray(x, dtype=np.float32)
    w_qkv = np.asarray(w_qkv, dtype=np.float32)
    b_qkv = np.asarray(b_qkv, dtype=np.float32)
    w_out = np.asarray(w_out, dtype=np.float32)
    b_out = np.asarray(b_out, dtype=np.float32)
    out, _ = run(x, w_qkv, b_qkv, w_out, b_out)
    return out


# revision 22
# speedup vs baseline: 1.0219x; 1.0219x over previous
"""Trainium2 Bass kernel for LocalWindowAttention.

Reference semantics (per batch b):
    pad seq 4000 -> 4096, split into 32 windows of 128 tokens.
    qkv = x @ w_qkv.T + b_qkv ; per-window per-head softmax(q k^T / sqrt(64)) @ v
    out = o @ w_out.T + b_out ; drop padded tail.

Sharding: data-parallel over batch. Core b computes batch b fully.

Per-core layout strategy (everything chosen so matmul contraction = partition dim):
  - x is staged feature-major  xT[e, t]  (e on partitions, 8 chunks of 128).
  - Q computed feature-major (f on partitions); K likewise but stored per-head
    zero-padded to the full 128 partitions (kz) so every score matmul reads
    inputs at base partition 0 (mixing base partitions 0/64 across matmuls
    crashes the runtime):
        S^T[tk, tq] = sum_d K[d, tk] Q[d, tq]   (lhsT=kz_h, rhs=Q pair, K=128)
  - V computed token-major (t on partitions) so AV works with V as stationary:
        O_u[d, tq] = sum_tk V[tk, d] E[tk, tq]
  - softmax denominators: 16 accumulating matmuls with one-hot selectors
        D16[h, tq] = sum_tk E[tk, h*128+tq]  -> reciprocal_approx_fast
    broadcast back to O shape via a (16 x 128) selector matmul, then one DVE
    multiply normalizes O.  (1/sqrt(64) is folded into w_q on the host; exp is
    computed without max-subtraction which is exact for softmax and safe here:
    |scores| <= ~3.)
  - out projection consumes O feature-major chunks directly.
All matmuls use bf16/fp16 operands (1 cycle/row on TRN2; fp32 is 4x slower).
Accumulation is always fp32 in PSUM.
"""

import sys
import numpy as np

for _p in ("/opt/trn_rl_repo", "/root/.axon_site/_ro/trn_rl_repo"):
    if _p not in sys.path:
        sys.path.append(_p)

import ml_dtypes

P = 128          # partitions
E = 1024         # embed dim
H = 16           # heads
D = 64           # head dim
W = 128          # window
B = 8            # batch
S = 4000         # seq len
SP = 4096        # padded seq len
NW = SP // W     # 32 windows
CW = 4           # windows per chunk
CT = CW * W      # 512 tokens per chunk
EC = 8           # e-chunks of 128

BF16 = ml_dtypes.bfloat16
F16 = np.float16

_cache = {}


def build_nc(n_chunks, s_out, has_bqk, has_bout):
    """Build + compile the single-core Bass program (same program for all cores)."""
    from concourse import bacc, tile, mybir

    dt = mybir.dt
    AF = mybir.ActivationFunctionType

    nc = bacc.Bacc(None, target_bir_lowering=False, debug=False)

    xt_d = nc.dram_tensor("xt", [n_chunks, P, EC, CT], dt.bfloat16, kind="ExternalInput")
    wqkv_d = nc.dram_tensor("wqkv", [P, EC, 3 * E], dt.bfloat16, kind="ExternalInput")
    wout_d = nc.dram_tensor("wout", [P, EC, E], dt.bfloat16, kind="ExternalInput")
    oh_d = nc.dram_tensor("onehot", [P, H, H], dt.bfloat16, kind="ExternalInput")
    sel_d = nc.dram_tensor("sel", [H, EC, P], dt.float16, kind="ExternalInput")
    out_d = nc.dram_tensor("out", [s_out, E], dt.float32, kind="ExternalOutput")
    if has_bqk:
        bqk_d = nc.dram_tensor("bqk", [P, 2, EC], dt.float32, kind="ExternalInput")
    if has_bout:
        cb_d = nc.dram_tensor("cb", [P, 2, 512], dt.float32, kind="ExternalInput")

    with tile.TileContext(nc) as tc:
        with (
            tc.tile_pool(name="const", bufs=1) as constp,
            tc.tile_pool(name="xp", bufs=2) as xp,
            tc.tile_pool(name="qkp", bufs=2) as qkp,
            tc.tile_pool(name="kzp", bufs=1) as kzp,
            tc.tile_pool(name="ktp", bufs=3) as ktp,
            tc.tile_pool(name="vp", bufs=2) as vp,
            tc.tile_pool(name="ep", bufs=2) as ep,
            tc.tile_pool(name="op", bufs=2) as opool,
            tc.tile_pool(name="rp", bufs=2) as rp,
            tc.tile_pool(name="fpl", bufs=3) as fpl,
            tc.tile_pool(name="psA", bufs=4, space="PSUM") as psA,
        ):
            oh = constp.tile([P, H, H], dt.bfloat16)
            nc.sync.dma_start(oh[:], oh_d[:])
            sel = constp.tile([H, EC, P], dt.float16)
            nc.sync.dma_start(sel[:], sel_d[:])
            # chunk 0's x arrives before the bulk of the weights so the first
            # QKV matmul only waits for xt[0] + wq[:, 0]
            xt_first = xp.tile([P, EC, CT], dt.bfloat16, tag="xt", name="xt_first")
            nc.sync.dma_start(xt_first[:], xt_d[0])
            # weights split per e-chunk so the first QKV matmul (which only
            # needs wq[:, 0]) isn't gated on the full 8.4MB transfer
            wq = constp.tile([P, EC, 3 * E], dt.bfloat16)
            for ec in range(EC):
                nc.sync.dma_start(wq[:, ec, :], wqkv_d[:, ec, :])
            wo = constp.tile([P, EC, E], dt.bfloat16)
            for ec in range(EC):
                nc.sync.dma_start(wo[:, ec, :], wout_d[:, ec, :])
            if has_bqk:
                bqk = constp.tile([P, 2, EC], dt.float32)
                nc.sync.dma_start(bqk[:], bqk_d[:])
            if has_bout:
                cb = constp.tile([P, 2, 512], dt.float32)
                nc.sync.dma_start(cb[:], cb_d[:])

            # kz zero halves never change: clear the two persistent tiles once.
            kz_tiles = []
            for i in range(2):
                kzt = kzp.tile([P, H, CT], dt.bfloat16, tag=f"kz{i}", name=f"kz{i}")
                nc.gpsimd.memset(kzt[:], 0.0)
                kz_tiles.append(kzt)

            def stage_a1(wi, kz_sb, q_sb):
                """scores -> exp (quartered ACTs so D16 can start early)."""
                e_sb = ep.tile([P, H, W], dt.bfloat16, tag="e")
                for half in range(2):
                    ps_s = psA.tile([P, 8, W], dt.float32, tag="ps")
                    for j in range(8):
                        h = half * 8 + j
                        # kz's invalid half is zero, so contracting all 128
                        # rows against the Q f-tile pair selects head h.
                        nc.tensor.matmul(
                            ps_s[:, j, :],
                            kz_sb[:, h, wi * W:(wi + 1) * W],
                            q_sb[:, h // 2, wi * W:(wi + 1) * W],
                            start=True,
                            stop=True,
                        )
                    for qq in range(2):
                        nc.scalar.activation(
                            e_sb[:, half * 8 + qq * 4:half * 8 + qq * 4 + 4, :],
                            ps_s[:, qq * 4:qq * 4 + 4, :], AF.Exp,
                        )
                return e_sb

            def stage_a2_d16(e_sb):
                """denominators D16[h, tq] via accumulating one-hot matmuls."""
                ps_d = psA.tile([H, W], dt.float32, tag="ps")
                for h in range(H):
                    nc.tensor.matmul(
                        ps_d[:],
                        oh[:, h, :],
                        e_sb[:, h, :],
                        start=(h == 0),
                        stop=(h == H - 1),
                    )
                return ps_d

            def stage_a2(wi, ps_d, e_sb, v_sb):
                """recip -> broadcast -> AV -> normalized O."""
                rd32 = rp.tile([H, W], dt.float32, tag="rd32")
                nc.vector.reciprocal_approx_fast(rd32[:], ps_d[:])
                rd16 = rp.tile([H, W], dt.float16, tag="rd16")
                nc.vector.tensor_copy(rd16[:], rd32[:])

                # broadcast recip to O shape: R_O[cc*128+p, tq] = rd[2cc+p//64, tq]
                ps_r = psA.tile([P, EC, W], dt.float32, tag="ps")
                for cc in range(EC):
                    nc.tensor.matmul(
                        ps_r[:, cc, :], sel[:, cc, :], rd16[:],
                        start=True, stop=True,
                    )
                # evict R_O to sbuf right away (runs during the AV matmuls, so
                # the post-AV critical chain is just one multiply per half)
                r_sb = rp.tile([P, EC, W], dt.float16, tag="ro")
                nc.vector.tensor_copy(r_sb[:], ps_r[:])

                # unnormalized O_u[d, tq] per head (2 heads per 128-row chunk)
                ps_o = psA.tile([P, EC, W], dt.float32, tag="ps")
                for h in range(H):
                    cc = h // 2
                    po = (h % 2) * D
                    nc.tensor.matmul(
                        ps_o[po:po + D, cc, :],
                        v_sb[:, wi, h // 8, (h % 8) * D:(h % 8) * D + D],
                        e_sb[:, h, :],
                        start=True,
                        stop=True,
                    )
                # normalize in halves into separate tiles so the out-projection
                # can start as soon as the first half is multiplied
                o_halves = []
                for hh in range(2):
                    o_h = opool.tile([P, 4, W], dt.bfloat16, tag=f"o{hh}",
                                     name=f"o{hh}")
                    sl = slice(hh * 4, hh * 4 + 4)
                    nc.vector.tensor_mul(o_h[:], ps_o[:, sl, :], r_sb[:, sl, :])
                    o_halves.append(o_h)
                return o_halves

            def stage_b_mm(ps_f, o_halves, fh):
                """one half of the out projection accumulation"""
                for cc in range(EC):
                    nc.tensor.matmul(
                        ps_f[:, fh, :],
                        o_halves[cc // 4][:, cc % 4, :],
                        wo[:, cc, fh * 512:(fh + 1) * 512],
                        start=(cc == 0),
                        stop=(cc == EC - 1),
                    )

            def stage_b_out(ps_f, row0, rows):
                f_sb = fpl.tile([P, 2, 512], dt.float32, tag="f")
                if has_bout:
                    nc.vector.tensor_add(f_sb[:], ps_f[:], cb[:])
                else:
                    # on ScalarE: keeps DVE free for the recip/normalize chain
                    nc.scalar.activation(f_sb[:], ps_f[:], AF.Copy)
                nc.sync.dma_start(out_d[row0:row0 + rows, :], f_sb[:rows])

            pend = None
            for c in range(n_chunks):
                if c == 0:
                    xt = xt_first
                else:
                    xt = xp.tile([P, EC, CT], dt.bfloat16, tag="xt")
                    nc.sync.dma_start(xt[:], xt_d[c])

                q_sb = qkp.tile([P, EC, CT], dt.bfloat16, tag="q")
                kz_sb = kz_tiles[c % 2]
                v_sb = vp.tile([P, CW, 2, 512], dt.bfloat16, tag="v")

                # ---- Q and K (feature-major): psum[f_tile, t] ----
                for which in (0, 1):
                    for fg in range(4):  # pairs of f-tiles -> one 2-bank psum tile
                        ps = psA.tile([P, 2, 512], dt.float32, tag="ps")
                        for half in range(2):
                            ft = fg * 2 + half
                            off = which * E + ft * P
                            for ec in range(EC):
                                nc.tensor.matmul(
                                    ps[:, half, :],
                                    wq[:, ec, off:off + P],
                                    xt[:, ec, :],
                                    start=(ec == 0),
                                    stop=(ec == EC - 1),
                                )
                        if which == 0:  # Q: keep f-tile-major pair layout
                            if has_bqk:
                                for half in range(2):
                                    ft = fg * 2 + half
                                    nc.scalar.activation(
                                        q_sb[:, ft, :], ps[:, half, :], AF.Identity,
                                        bias=bqk[:, 0, ft:ft + 1],
                                    )
                            else:
                                nc.scalar.activation(
                                    q_sb[:, fg * 2:fg * 2 + 2, :], ps[:], AF.Copy,
                                )
                        else:  # K: evict pair once, DMA-scatter into kz halves
                            ktmp = ktp.tile([P, 2, 512], dt.bfloat16, tag="kt")
                            if has_bqk:
                                for half in range(2):
                                    ft = fg * 2 + half
                                    nc.scalar.activation(
                                        ktmp[:, half, :], ps[:, half, :],
                                        AF.Identity, bias=bqk[:, 1, ft:ft + 1],
                                    )
                            else:
                                nc.scalar.activation(ktmp[:], ps[:], AF.Copy)
                            for half in range(2):
                                ft = fg * 2 + half
                                for hh in range(2):
                                    pr = slice(hh * 64, hh * 64 + 64)
                                    nc.sync.dma_start(
                                        kz_sb[pr, 2 * ft + hh, :],
                                        ktmp[pr, half, :],
                                    )

                # ---- V (token-major): psum[t, f] per window ----
                for wi in range(CW):
                    ps = psA.tile([P, 2, 512], dt.float32, tag="ps")
                    for fh in range(2):
                        off = 2 * E + fh * 512
                        for ec in range(EC):
                            nc.tensor.matmul(
                                ps[:, fh, :],
                                xt[:, ec, wi * W:(wi + 1) * W],
                                wq[:, ec, off:off + 512],
                                start=(ec == 0),
                                stop=(ec == EC - 1),
                            )
                    nc.vector.tensor_copy(v_sb[:, wi], ps[:])

                # ---- attention (A) + out-projection (B), software-pipelined:
                # B(w) is emitted after A(w+1) so the PE has score/AV matmuls
                # to run while w's evict->normalize chain goes through ACT/DVE.
                for wi in range(CW):
                    g = c * CW + wi
                    row0 = g * W
                    rows = min(s_out - row0, W)
                    if rows <= 0:
                        continue
                    e_sb = stage_a1(wi, kz_sb, q_sb)
                    if pend is not None:  # outproj half 0 of w-1 covers exp(w)
                        ps_f = psA.tile([P, 2, 512], dt.float32, tag="ps")
                        stage_b_mm(ps_f, pend[0], 0)
                    ps_d = stage_a2_d16(e_sb)
                    if pend is not None:  # half 1 covers recip/cast chain
                        stage_b_mm(ps_f, pend[0], 1)
                        stage_b_out(ps_f, pend[1], pend[2])
                    o_halves = stage_a2(wi, ps_d, e_sb, v_sb)
                    pend = (o_halves, row0, rows)

            if pend is not None:
                ps_f = psA.tile([P, 2, 512], dt.float32, tag="ps")
                stage_b_mm(ps_f, pend[0], 0)
                stage_b_mm(ps_f, pend[0], 1)
                stage_b_out(ps_f, pend[1], pend[2])

    nc.compile()
    return nc


def prep_inputs(x, w_qkv, b_qkv, w_out, b_out, n_chunks, s_out):
    """Host-side staging: pad, transpose, cast, fold scale into w_q."""
    sp = n_chunks * CT
    nb = x.shape[0]

    wqkvT = np.ascontiguousarray(w_qkv.T).astype(np.float32).copy()
    wqkvT[:, :E] *= 1.0 / np.sqrt(D)
    wqkv_sb = np.ascontiguousarray(
        wqkvT.reshape(EC, P, 3 * E).transpose(1, 0, 2)
    ).astype(BF16)

    woutT = np.ascontiguousarray(w_out.T)
    wout_sb = np.ascontiguousarray(
        woutT.reshape(EC, P, E).transpose(1, 0, 2)
    ).astype(BF16)

    oh = np.zeros((P, H, H), dtype=BF16)
    for h in range(H):
        oh[:, h, h] = 1
    selm = np.zeros((H, EC, P), dtype=F16)
    for cc in range(EC):
        for m in range(P):
            selm[2 * cc + m // D, cc, m] = 1

    base = {"wqkv": wqkv_sb, "wout": wout_sb, "onehot": oh, "sel": selm}

    has_bqk = bool(np.any(b_qkv[:2 * E]))
    has_bout = bool(np.any(b_out)) or bool(np.any(b_qkv[2 * E:]))
    if has_bqk:
        bqk = np.stack(
            [b_qkv[:E].reshape(EC, P).T / np.sqrt(D),
             b_qkv[E:2 * E].reshape(EC, P).T], axis=1
        ).astype(np.float32)  # (P, 2, EC)
        base["bqk"] = np.ascontiguousarray(bqk)
    if has_bout:
        cbv = (b_out + b_qkv[2 * E:] @ w_out.T).astype(np.float32)  # (E,)
        base["cb"] = np.ascontiguousarray(
            np.broadcast_to(cbv.reshape(1, 2, 512), (P, 2, 512))
        ).copy()

    in_maps = []
    for b in range(nb):
        xp_ = np.zeros((sp, E), dtype=np.float32)
        xp_[:min(s_out, x.shape[1])] = x[b][:s_out]
        xT = np.ascontiguousarray(xp_.T)  # (E, sp)
        xt_sb = np.ascontiguousarray(
            xT.reshape(EC, P, n_chunks, CT).transpose(2, 1, 0, 3)
        ).astype(BF16)  # (n_chunks, P, EC, CT)
        m = dict(base)
        m["xt"] = xt_sb
        in_maps.append(m)
    return in_maps, has_bqk, has_bout


def run(x, w_qkv, b_qkv, w_out, b_out, n_chunks=NW // CW, s_out=S, trace=False):
    from concourse import bass_utils

    in_maps, has_bqk, has_bout = prep_inputs(
        x, w_qkv, b_qkv, w_out, b_out, n_chunks, s_out
    )
    key = (n_chunks, s_out, has_bqk, has_bout)
    if key not in _cache:
        _cache[key] = build_nc(*key)
    nc = _cache[key]

    res = bass_utils.run_bass_kernel_spmd(
        nc, in_maps, core_ids=list(range(len(in_maps))), trace=trace,
    )
    out = np.stack([r["out"] for r in res.results], axis=0)
    return out, res


def kernel(x, w_qkv, b_qkv, w_out, b_out):
    x = np.asarray(x, dtype=np.float32)
    w_qkv = np.asarray(w_qkv, dtype=np.float32)
    b_qkv = np.asarray(b_qkv, dtype=np.float32)
    w_out = np.asarray(w_out, dtype=np.float32)
    b_out = np.asarray(b_out, dtype=np.float32)
    out, _ = run(x, w_qkv, b_qkv, w_out, b_out)
    return out
